# revision 15
# baseline (speedup 1.0000x reference)
"""CharRNN forward (fully parallel over T) on 8 trn2 NeuronCores.

reference:
    hcontrib = hprev @ Whh + bh            # [1, H]
    h  = tanh(x @ Wxh + hcontrib)          # [T, H]
    logits = h @ Why + by                  # [T, V]
    return logits, h[-1:]

Sharding: data-parallel over T (T=4096 -> 512 per core); Wxh/Why replicated.
Host prep: cast to bf16, transpose x -> xT [V, T] so the contracted vocab dim
lands on SBUF partitions with contiguous DMA lines; hcontrib ([1,H], 8 MFLOP)
computed on host in fp32.

Device (per core):
  phase 1: hT[hid, t] = tanh(Wxh.T @ x.T + hcontrib)
           lhsT = Wxh tiles (natural [V, H] layout), rhs = xT tiles.
           16 hid-tiles of [128, TL]; 8 PSUM banks -> 2 halves; tanh + per-
           partition bias fused into the PSUM->SBUF eviction on ScalarE.
           hT stays resident in SBUF (2 MB) in exactly the layout matmul 2
           needs for its stationary operand.
  phase 2: logits[t, v] = hT.T @ Why, lhsT = hT slices, rhs = Why tiles
           (natural [H, V] layout), fp32 eviction on VectorE, DMA out.
"""

from contextlib import ExitStack

import numpy as np

import concourse.bacc as bacc
import concourse.mybir as mybir
import concourse.tile as tile
from concourse.bass_utils import run_bass_kernel_spmd

P = 128
N_CORES = 8
BF16 = mybir.dt.np(mybir.dt.bfloat16)

# problem dims (hardcoded per spec)
V = 32000
H = 2048
T = 4096


def build_nc(v, h, tl, nw=500, x_bufs=10, w_bufs=10, wy_bufs=3, out_bufs=8):
    """Build the per-core Bass program.

    v: vocab (contraction dim of mm1, free dim of mm2)
    h: hidden (free dim of mm1, contraction dim of mm2)
    tl: local timesteps on this core
    nw: free-dim tile width for mm2 (must divide v, <= 512)
    """
    assert v % P == 0 and h % P == 0 and v % nw == 0 and nw <= 512
    KV = v // P          # k-tiles of mm1
    MH = h // P          # hid tiles
    KH = h // P          # k-tiles of mm2
    NT = v // nw         # n-tiles of mm2
    TSUB = tl // P       # t subtiles of mm2 output
    assert tl % P == 0 and tl <= 512
    assert MH % 2 == 0
    HALF = MH // 2

    nc = bacc.Bacc(
        "TRN2",
        target_bir_lowering=False,
        debug=False,
        num_devices=N_CORES,
    )

    xT = nc.dram_tensor("xT", [v, tl], mybir.dt.bfloat16, kind="ExternalInput").ap()
    wxh = nc.dram_tensor("wxh", [v, h], mybir.dt.bfloat16, kind="ExternalInput").ap()
    why = nc.dram_tensor("why", [h, v], mybir.dt.bfloat16, kind="ExternalInput").ap()
    hc = nc.dram_tensor("hc", [P, MH], mybir.dt.float32, kind="ExternalInput").ap()
    logits = nc.dram_tensor(
        "logits", [tl, v], mybir.dt.float32, kind="ExternalOutput"
    ).ap()
    hlast = nc.dram_tensor(
        "hlast", [P, MH], mybir.dt.float32, kind="ExternalOutput"
    ).ap()

    why_r = why.rearrange("(kt p) v -> p kt v", p=P)  # [P, KH, v]

    with tile.TileContext(nc) as tc, ExitStack() as ctx:
        const = ctx.enter_context(tc.tile_pool(name="const", bufs=1))
        hcsb = const.tile([P, MH], mybir.dt.float32, name="hc", tag="hc")
        nc.sync.dma_start(hcsb[:], hc[:])
        hlsb = const.tile([P, MH], mybir.dt.float32, name="hlsb", tag="hlast")

        hpool = ctx.enter_context(tc.tile_pool(name="hpool", bufs=1))
        xpool = ctx.enter_context(tc.tile_pool(name="xpool", bufs=x_bufs))
        wpool = ctx.enter_context(tc.tile_pool(name="wpool", bufs=w_bufs))
        wypool = ctx.enter_context(tc.tile_pool(name="wypool", bufs=wy_bufs))
        opool = ctx.enter_context(tc.tile_pool(name="opool", bufs=out_bufs))
        psum = ctx.enter_context(tc.tile_pool(name="psum", bufs=8, space="PSUM"))

        # PE pre-warm: tiny matmuls on a zeroed tile fill the otherwise-idle
        # initial DMA wait so the HAM clock-gate opens (1.2 -> 2.4 GHz)
        # before the real matmul stream begins.
        warm = const.tile([P, 32], mybir.dt.bfloat16, name="warm", tag="warm")
        nc.any.memset(warm[:], 0.0)
        wps = psum.tile([P, 512], mybir.dt.float32, name="wps", tag="ps")
        for _ in range(176):
            nc.tensor.matmul(
                wps[:32, :32], warm[:, :32], warm[:, :32], start=True, stop=True
            )

        # ---------------- phase 1: hT = tanh(Wxh.T @ x.T + hc) ----------------
        ht = []
        p1_marks = []  # progress markers used to hold back phase-2 prefetch
        for half in range(2):
            ps = [psum.tile([P, 512], mybir.dt.float32, name="ps1", tag="ps") for _ in range(HALF)]
            for k in range(KV):
                xk = xpool.tile([P, tl], mybir.dt.bfloat16, name="xk", tag="xk")
                nc.sync.dma_start(xk[:], xT[k * P : (k + 1) * P, :])
                wk = wpool.tile([P, HALF * P], mybir.dt.bfloat16, name="wk", tag="wk")
                wsrc = wxh[k * P : (k + 1) * P, half * HALF * P : (half + 1) * HALF * P]
                if half == 0 and k == 0:
                    # split the very first weight tile into per-m chunks: early
                    # DMAs drain serially, and the first matmul only needs
                    # xk + the m=0 slice -- starts ~1.3us sooner
                    for m in range(HALF):
                        nc.sync.dma_start(
                            wk[:, m * P : (m + 1) * P], wsrc[:, m * P : (m + 1) * P]
                        )
                else:
                    nc.sync.dma_start(wk[:], wsrc)
                for m in range(HALF):
                    mm = nc.tensor.matmul(
                        ps[m][:, :tl],
                        wk[:, m * P : (m + 1) * P],
                        xk[:],
                        start=(k == 0),
                        stop=(k == KV - 1),
                    )
                    if half == 1 and m == 0 and k in (0, KV // 4, KV // 2):
                        p1_marks.append(mm)
            # interleave the tiny last-timestep eviction with each ht tile so
            # every PSUM slot frees ~immediately after its ht eviction --
            # deferring them to the end of the half delays slot recycling and
            # opens a ~3.5us PE gap (plus a HAM re-throttle) at the boundary
            for m in range(HALF):
                mi = half * HALF + m
                t = hpool.tile([P, tl], mybir.dt.bfloat16, name=f"ht{mi}", tag=f"ht{mi}")
                nc.scalar.activation(
                    t[:],
                    ps[m][:, :tl],
                    mybir.ActivationFunctionType.Tanh,
                    bias=hcsb[:, mi : mi + 1],
                )
                ht.append(t)
                nc.scalar.activation(
                    hlsb[:, mi : mi + 1],
                    ps[m][:, tl - 1 : tl],
                    mybir.ActivationFunctionType.Tanh,
                    bias=hcsb[:, mi : mi + 1],
                )
        nc.sync.dma_start(hlast[:], hlsb[:])

        # ---------------- phase 2: logits = hT.T @ Why ----------------
        for n in range(NT):
            wy = wypool.tile([P, KH, nw], mybir.dt.bfloat16, name="wy", tag="wy")
            wydma = nc.sync.dma_start(wy[:], why_r[:, :, n * nw : (n + 1) * nw])
            if n < len(p1_marks):
                # hold phase-2 weight prefetch back until phase 1 is well
                # underway -- the 2MB transfers otherwise starve the
                # startup x/Wxh DMAs and delay the first matmul by ~10us
                tile.add_dep_helper(wydma.ins, p1_marks[n].ins, sync=True)
            for m in range(TSUB):
                po = psum.tile([P, 512], mybir.dt.float32, name="ps2", tag="ps")
                for k in range(KH):
                    nc.tensor.matmul(
                        po[:, :nw],
                        ht[k][:, m * P : (m + 1) * P],
                        wy[:, k, :],
                        start=(k == 0),
                        stop=(k == KH - 1),
                    )
                ot = opool.tile([P, nw], mybir.dt.float32, name="ot", tag="ot")
                if n == NT - 1 and m == TSUB - 1:
                    # split the final eviction: the second half's copy + DMA
                    # are all that serialize between the last matmul and the
                    # kernel-tail barrier
                    hw = nw // 2
                    for s0, s1 in ((0, hw), (hw, nw)):
                        nc.vector.tensor_copy(ot[:, s0:s1], po[:, s0:s1])
                        nc.sync.dma_start(
                            logits[m * P : (m + 1) * P, n * nw + s0 : n * nw + s1],
                            ot[:, s0:s1],
                        )
                else:
                    nc.vector.tensor_copy(ot[:], po[:, :nw])
                    nc.sync.dma_start(
                        logits[m * P : (m + 1) * P, n * nw : (n + 1) * nw], ot[:]
                    )

    nc.compile()
    return nc


_NC_CACHE = {}

# test-harness hooks: set TRACE=True before calling kernel() to capture an
# NTFF profile; the full BassKernelResults of the last run lands in
# LAST_RESULTS (exec_time_ns etc.).
TRACE = False
LAST_RESULTS = None


def _get_nc(v, h, tl):
    key = (v, h, tl)
    if key not in _NC_CACHE:
        _NC_CACHE[key] = build_nc(v, h, tl)
    return _NC_CACHE[key]


def kernel(hprev, x, Wxh, Whh, Why, bh, by):
    hprev = np.asarray(hprev, dtype=np.float32)
    x = np.asarray(x, dtype=np.float32)
    Wxh = np.asarray(Wxh, dtype=np.float32)
    Whh = np.asarray(Whh, dtype=np.float32)
    Why = np.asarray(Why, dtype=np.float32)
    bh = np.asarray(bh, dtype=np.float32)
    by = np.asarray(by, dtype=np.float32)

    t, v = x.shape
    hdim = Whh.shape[0]
    tl = t // N_CORES
    mh = hdim // P

    nc = _get_nc(v, hdim, tl)

    # host prep: tiny hcontrib in fp32, bf16 casts, x transpose
    hcontrib = (hprev @ Whh + bh).reshape(-1)              # [H]
    hc_dev = np.ascontiguousarray(hcontrib.reshape(mh, P).T)  # [P, MH]
    xT = np.ascontiguousarray(x.astype(BF16).T)            # [V, T]
    wxh_dev = Wxh.astype(BF16)
    why_dev = Why.astype(BF16)

    in_maps = [
        {
            "xT": np.ascontiguousarray(xT[:, c * tl : (c + 1) * tl]),
            "wxh": wxh_dev,
            "why": why_dev,
            "hc": hc_dev,
        }
        for c in range(N_CORES)
    ]

    global LAST_RESULTS
    LAST_RESULTS = run_bass_kernel_spmd(
        nc, in_maps, core_ids=list(range(N_CORES)), trace=TRACE
    )
    results = LAST_RESULTS.results

    logits = np.concatenate([r["logits"] for r in results], axis=0)
    if by.any():
        logits = logits + by[None, :]
    hl = results[-1]["hlast"]                              # [P, MH]
    h_last = np.ascontiguousarray(hl.T).reshape(1, hdim)
    return logits, h_last


# revision 18
# speedup vs baseline: 1.0045x; 1.0045x over previous
"""CharRNN forward (fully parallel over T) on 8 trn2 NeuronCores.

reference:
    hcontrib = hprev @ Whh + bh            # [1, H]
    h  = tanh(x @ Wxh + hcontrib)          # [T, H]
    logits = h @ Why + by                  # [T, V]
    return logits, h[-1:]

Sharding: data-parallel over T (T=4096 -> 512 per core); Wxh/Why replicated.
Host prep: cast to bf16, transpose x -> xT [V, T] so the contracted vocab dim
lands on SBUF partitions with contiguous DMA lines; hcontrib ([1,H], 8 MFLOP)
computed on host in fp32.

Device (per core):
  phase 1: hT[hid, t] = tanh(Wxh.T @ x.T + hcontrib)
           lhsT = Wxh tiles (natural [V, H] layout), rhs = xT tiles.
           16 hid-tiles of [128, TL]; 8 PSUM banks -> 2 halves; tanh + per-
           partition bias fused into the PSUM->SBUF eviction on ScalarE.
           hT stays resident in SBUF (2 MB) in exactly the layout matmul 2
           needs for its stationary operand.
  phase 2: logits[t, v] = hT.T @ Why, lhsT = hT slices, rhs = Why tiles
           (natural [H, V] layout), fp32 eviction on VectorE, DMA out.
"""

from contextlib import ExitStack

import numpy as np

import concourse.bacc as bacc
import concourse.mybir as mybir
import concourse.tile as tile
from concourse.bass_utils import run_bass_kernel_spmd

P = 128
N_CORES = 8
BF16 = mybir.dt.np(mybir.dt.bfloat16)

# problem dims (hardcoded per spec)
V = 32000
H = 2048
T = 4096


def build_nc(v, h, tl, nw=500, x_bufs=10, w_bufs=10, wy_bufs=3, out_bufs=8):
    """Build the per-core Bass program.

    v: vocab (contraction dim of mm1, free dim of mm2)
    h: hidden (free dim of mm1, contraction dim of mm2)
    tl: local timesteps on this core
    nw: free-dim tile width for mm2 (must divide v, <= 512)
    """
    assert v % P == 0 and h % P == 0 and v % nw == 0 and nw <= 512
    KV = v // P          # k-tiles of mm1
    MH = h // P          # hid tiles
    KH = h // P          # k-tiles of mm2
    NT = v // nw         # n-tiles of mm2
    TSUB = tl // P       # t subtiles of mm2 output
    assert tl % P == 0 and tl <= 512
    assert MH % 2 == 0
    HALF = MH // 2

    nc = bacc.Bacc(
        "TRN2",
        target_bir_lowering=False,
        debug=False,
        num_devices=N_CORES,
    )

    xT = nc.dram_tensor("xT", [v, tl], mybir.dt.bfloat16, kind="ExternalInput").ap()
    wxh = nc.dram_tensor("wxh", [v, h], mybir.dt.bfloat16, kind="ExternalInput").ap()
    why = nc.dram_tensor("why", [h, v], mybir.dt.bfloat16, kind="ExternalInput").ap()
    hc = nc.dram_tensor("hc", [P, MH], mybir.dt.float32, kind="ExternalInput").ap()
    logits = nc.dram_tensor(
        "logits", [tl, v], mybir.dt.float32, kind="ExternalOutput"
    ).ap()
    hlast = nc.dram_tensor(
        "hlast", [P, MH], mybir.dt.float32, kind="ExternalOutput"
    ).ap()

    why_r = why.rearrange("(kt p) v -> p kt v", p=P)  # [P, KH, v]

    with tile.TileContext(nc) as tc, ExitStack() as ctx:
        const = ctx.enter_context(tc.tile_pool(name="const", bufs=1))
        hcsb = const.tile([P, MH], mybir.dt.float32, name="hc", tag="hc")
        nc.sync.dma_start(hcsb[:], hc[:])
        hlsb = const.tile([P, MH], mybir.dt.float32, name="hlsb", tag="hlast")

        hpool = ctx.enter_context(tc.tile_pool(name="hpool", bufs=1))
        xpool = ctx.enter_context(tc.tile_pool(name="xpool", bufs=x_bufs))
        wpool = ctx.enter_context(tc.tile_pool(name="wpool", bufs=w_bufs))
        wypool = ctx.enter_context(tc.tile_pool(name="wypool", bufs=wy_bufs))
        opool = ctx.enter_context(tc.tile_pool(name="opool", bufs=out_bufs))
        psum = ctx.enter_context(tc.tile_pool(name="psum", bufs=8, space="PSUM"))

        # PE pre-warm: tiny matmuls on a zeroed tile fill the otherwise-idle
        # initial DMA wait so the HAM clock-gate opens (1.2 -> 2.4 GHz)
        # before the real matmul stream begins.
        warm = const.tile([P, 32], mybir.dt.bfloat16, name="warm", tag="warm")
        nc.any.memset(warm[:], 0.0)
        wps = psum.tile([P, 512], mybir.dt.float32, name="wps", tag="ps")
        for _ in range(200):
            nc.tensor.matmul(
                wps[:32, :32], warm[:, :32], warm[:, :32], start=True, stop=True
            )

        # ---------------- phase 1: hT = tanh(Wxh.T @ x.T + hc) ----------------
        ht = []
        p1_marks = []  # progress markers used to hold back phase-2 prefetch
        for half in range(2):
            ps = [psum.tile([P, 512], mybir.dt.float32, name="ps1", tag="ps") for _ in range(HALF)]
            for k in range(KV):
                # wk before xk: the first matmul needs both, and early DMAs
                # drain serially through one queue -- put the bigger one first
                wk = wpool.tile([P, HALF * P], mybir.dt.bfloat16, name="wk", tag="wk")
                nc.sync.dma_start(
                    wk[:],
                    wxh[k * P : (k + 1) * P, half * HALF * P : (half + 1) * HALF * P],
                )
                xk = xpool.tile([P, tl], mybir.dt.bfloat16, name="xk", tag="xk")
                nc.sync.dma_start(xk[:], xT[k * P : (k + 1) * P, :])
                for m in range(HALF):
                    mm = nc.tensor.matmul(
                        ps[m][:, :tl],
                        wk[:, m * P : (m + 1) * P],
                        xk[:],
                        start=(k == 0),
                        stop=(k == KV - 1),
                    )
                    if half == 1 and m == 0 and k in (0, KV // 4, KV // 2):
                        p1_marks.append(mm)
            # interleave the tiny last-timestep eviction with each ht tile so
            # every PSUM slot frees ~immediately after its ht eviction --
            # deferring them to the end of the half delays slot recycling and
            # opens a ~3.5us PE gap (plus a HAM re-throttle) at the boundary
            for m in range(HALF):
                mi = half * HALF + m
                t = hpool.tile([P, tl], mybir.dt.bfloat16, name=f"ht{mi}", tag=f"ht{mi}")
                nc.scalar.activation(
                    t[:],
                    ps[m][:, :tl],
                    mybir.ActivationFunctionType.Tanh,
                    bias=hcsb[:, mi : mi + 1],
                )
                ht.append(t)
                nc.scalar.activation(
                    hlsb[:, mi : mi + 1],
                    ps[m][:, tl - 1 : tl],
                    mybir.ActivationFunctionType.Tanh,
                    bias=hcsb[:, mi : mi + 1],
                )
        nc.sync.dma_start(hlast[:], hlsb[:])

        # ---------------- phase 2: logits = hT.T @ Why ----------------
        for n in range(NT):
            wy = wypool.tile([P, KH, nw], mybir.dt.bfloat16, name="wy", tag="wy")
            wydma = nc.sync.dma_start(wy[:], why_r[:, :, n * nw : (n + 1) * nw])
            if n < len(p1_marks):
                # hold phase-2 weight prefetch back until phase 1 is well
                # underway -- the 2MB transfers otherwise starve the
                # startup x/Wxh DMAs and delay the first matmul by ~10us
                tile.add_dep_helper(wydma.ins, p1_marks[n].ins, sync=True)
            for m in range(TSUB):
                po = psum.tile([P, 512], mybir.dt.float32, name="ps2", tag="ps")
                for k in range(KH):
                    nc.tensor.matmul(
                        po[:, :nw],
                        ht[k][:, m * P : (m + 1) * P],
                        wy[:, k, :],
                        start=(k == 0),
                        stop=(k == KH - 1),
                    )
                ot = opool.tile([P, nw], mybir.dt.float32, name="ot", tag="ot")
                nc.vector.tensor_copy(ot[:], po[:, :nw])
                nc.sync.dma_start(
                    logits[m * P : (m + 1) * P, n * nw : (n + 1) * nw], ot[:]
                )

    nc.compile()
    return nc


_NC_CACHE = {}

# test-harness hooks: set TRACE=True before calling kernel() to capture an
# NTFF profile; the full BassKernelResults of the last run lands in
# LAST_RESULTS (exec_time_ns etc.).
TRACE = False
LAST_RESULTS = None


def _get_nc(v, h, tl):
    key = (v, h, tl)
    if key not in _NC_CACHE:
        _NC_CACHE[key] = build_nc(v, h, tl)
    return _NC_CACHE[key]


def kernel(hprev, x, Wxh, Whh, Why, bh, by):
    hprev = np.asarray(hprev, dtype=np.float32)
    x = np.asarray(x, dtype=np.float32)
    Wxh = np.asarray(Wxh, dtype=np.float32)
    Whh = np.asarray(Whh, dtype=np.float32)
    Why = np.asarray(Why, dtype=np.float32)
    bh = np.asarray(bh, dtype=np.float32)
    by = np.asarray(by, dtype=np.float32)

    t, v = x.shape
    hdim = Whh.shape[0]
    tl = t // N_CORES
    mh = hdim // P

    nc = _get_nc(v, hdim, tl)

    # host prep: tiny hcontrib in fp32, bf16 casts, x transpose
    hcontrib = (hprev @ Whh + bh).reshape(-1)              # [H]
    hc_dev = np.ascontiguousarray(hcontrib.reshape(mh, P).T)  # [P, MH]
    xT = np.ascontiguousarray(x.astype(BF16).T)            # [V, T]
    wxh_dev = Wxh.astype(BF16)
    why_dev = Why.astype(BF16)

    in_maps = [
        {
            "xT": np.ascontiguousarray(xT[:, c * tl : (c + 1) * tl]),
            "wxh": wxh_dev,
            "why": why_dev,
            "hc": hc_dev,
        }
        for c in range(N_CORES)
    ]

    global LAST_RESULTS
    LAST_RESULTS = run_bass_kernel_spmd(
        nc, in_maps, core_ids=list(range(N_CORES)), trace=TRACE
    )
    results = LAST_RESULTS.results

    logits = np.concatenate([r["logits"] for r in results], axis=0)
    if by.any():
        logits = logits + by[None, :]
    hl = results[-1]["hlast"]                              # [P, MH]
    h_last = np.ascontiguousarray(hl.T).reshape(1, hdim)
    return logits, h_last
